# revision 30
# baseline (speedup 1.0000x reference)
"""Causal multi-head attention (B=2, S=2048, H=32, D=128) on 8 TRN2 NeuronCores.

Strategy (tensor-parallel over (batch, head) pairs — 64 pairs, 8 per core):

Host side packs per-head inputs into device-friendly layouts:
  qT, kT : [hpc, D, S]  bf16 — Q^T / K^T per head (d on partitions)
  vA     : [hpc, 128, NT*129] bf16 — V tiled [kv-tile, 129] with a ones
           column appended (col 128) so the softmax denominator falls out of
           the PV matmul as an extra output column.
  tri    : [128, 128] bf16 — tri[p, f] = 1 iff p <= f (causal keep-mask for
           diagonal 128x128 blocks in S^T layout).

Device per head (q split into blocks of [512, 768, 768] columns):
  S^T[kv, q] pieces = K_tile^T-weights @ Q^T (PE, bf16, fp32 PSUM), causal-
  trimmed per kv-tile and bank-packed into 3-bank PSUM waves; one big exp
  per wave on ACT (scale=1/sqrt(D) folded in, no max subtraction — scores
  are O(5) so exp is safe in fp32); causal diagonal fixed by a bf16 tri-mask
  multiply on DVE; PV with P^T chunks as the stationary operand so the
  output lands in natural [q, d] layout and the ones column of vA
  accumulates the row sums; normalize with reciprocal + tensor_scalar on
  DVE.

Upper-triangle blocks are skipped entirely: exp(-1e9) underflows to exactly
0.0 in fp32, so dropping them is bit-equivalent to the reference softmax.
"""

import math

import numpy as np
import ml_dtypes

import concourse.bass as bass
import concourse.mybir as mybir
import concourse.tile as tile
from concourse import bacc
from concourse.tile_rust import add_dep_helper

B, S, H, D = 2, 2048, 32, 128
N_CORES = 8
HPC = (B * H) // N_CORES  # head-pairs per core
VW = D + 1                # V width including the ones column
SCALE = 1.0 / math.sqrt(D)
BF16 = mybir.dt.bfloat16
F32 = mybir.dt.float32

# q-blocks per head, largest first (deep PE work at head starts, small tail).
# Each block's PV accumulator fits 2 PSUM banks: <=3 chunks of 129 per bank.
BLOCKS = [(1536, 512), (768, 768), (0, 768)]  # (q0, qw)
CHUNK_OFF = (0, 129, 258, 512, 641, 770)


def _block_layout(q0, qw):
    """Bank-packed S^T piece layout for q-block [q0, q0+qw).

    Returns (pieces, nbanks, valid) where pieces is a list of
    (j, qs, w, col): kv-tile j's scores for q columns [qs, qs+w) land at
    packed column `col`. Pieces are causal-trimmed, <=512 wide, and packed
    so none crosses a 512-col PSUM bank boundary, with all slack trailing.
    """
    jmax = (q0 + qw) // 128
    raw = []
    for j in range(jmax):
        qs = max(q0, j * 128)
        t = q0 + qw - qs
        while t > 512:
            raw.append((j, qs, 512))
            qs += 512
            t -= 512
        raw.append((j, qs, t))
    by = {4: [], 3: [], 2: [], 1: []}
    for p in raw:
        by[p[2] // 128].append(p)
    order = list(by[4])
    while by[3] and by[1]:
        order.append(by[3].pop(0))
        order.append(by[1].pop(0))
    while len(by[2]) >= 2:
        order.append(by[2].pop(0))
        order.append(by[2].pop(0))
    while by[2] and len(by[1]) >= 2:
        order.append(by[2].pop(0))
        order.append(by[1].pop(0))
        order.append(by[1].pop(0))
    while len(by[1]) >= 4:
        for _ in range(4):
            order.append(by[1].pop(0))
    tail = by[3] + by[2] + by[1]  # trailing partial bank (slack at end)
    pieces = []
    col = 0
    for (j, qs, w) in order + tail:
        pieces.append((j, qs, w, col))
        col += w
    valid = col
    nbanks = (valid + 511) // 512
    return pieces, nbanks, valid


def build_module(hpc=HPC, s=S, wave_banks=3):
    nt = s // 128
    ptw = 15 * 512  # widest packed q-block (the 512-wide block, 16 kv-tiles)

    nc = bacc.Bacc(trn_type="TRN2")
    qT = nc.dram_tensor("qT", [hpc, D, s], BF16, kind="ExternalInput")
    kT = nc.dram_tensor("kT", [hpc, D, s], BF16, kind="ExternalInput")
    vA = nc.dram_tensor("vA", [hpc, 128, nt * VW], BF16, kind="ExternalInput")
    tri = nc.dram_tensor("tri", [128, 128], BF16, kind="ExternalInput")
    out = nc.dram_tensor("out", [hpc, 128, nt * D], F32, kind="ExternalOutput")

    exp_fn = mybir.ActivationFunctionType.Exp

    with tile.TileContext(nc) as tc:
        with (
            tc.tile_pool(name="const", bufs=1) as cpool,
            tc.tile_pool(name="io", bufs=2) as iopool,
            tc.tile_pool(name="pt", bufs=3) as ptpool,
            tc.tile_pool(name="ps", bufs=2, space="PSUM") as pspool,
            tc.tile_pool(name="po", bufs=1, space="PSUM") as popool,
            tc.tile_pool(name="nrm", bufs=4) as npool,
            tc.tile_pool(name="un", bufs=2) as unpool,
        ):
            tri_sb = cpool.tile([128, 128], BF16, tag="tri", name="tri_sb")

            # ---- PE warmup during the ~10us initial DMA window ----
            # Three dummy matmuls on a zeroed tile fill the pre-DMA idle so
            # the PE's HAM clock-gate busy streak starts ~2us earlier; the
            # count is small so they finish before real data arrives.
            wu_sb = cpool.tile([128, 512], BF16, tag="wu", name="wu_sb")
            nc.vector.memset(wu_sb, 0.0)
            ps_wu = pspool.tile([128, wave_banks * 512], F32, tag="ps",
                                name="ps_warm")
            for wi in range(3):
                nc.tensor.matmul(
                    ps_wu[:, 0:512], wu_sb[:, 0:128], wu_sb[:, 0:512],
                    start=True, stop=True,
                )

            # ---- flat wave pipeline across q-blocks and heads ----
            # Per wave: scores matmuls -> exp (ACT) -> diag tri-mask (DVE);
            # PV matmuls trail behind so PE streams wave w+1's scores while
            # ACT runs exp(w) and always has PV work queued.
            pending = []   # wave dicts awaiting PV emission (lag queue)
            PV_LAG = 2     # PV trails scores by 2 waves: its exp/tri deps are
                           # guaranteed complete (ps slot WAR), so PE never
                           # head-of-line blocks on ACT/DVE.
            PV_FIRST_EXTRA = 2  # extra delay for the first PV of a q-block:
                           # its start=True write waits on the previous
                           # block's po drain (DVE copies, ~1us), so queue
                           # two more scores waves in front as cover.

            def pv_lag_target():
                if pending and pending[0]["first"]:
                    return PV_LAG + PV_FIRST_EXTRA
                return PV_LAG

            def emit_scores(wv):
                st = wv["st"]
                ps = pspool.tile(
                    [128, wave_banks * 512], F32, tag="ps",
                    name=f"ps{wv['h']}_{wv['q0']}_{wv['wb']}",
                )
                for (j, qs, w, col) in wv["pieces"]:
                    lcol = col - wv["wb"] * 512
                    nc.tensor.matmul(
                        ps[:, lcol:lcol + w],
                        st["kT"][:, j * 128:(j + 1) * 128],
                        st["qT"][:, qs:qs + w],
                        start=True, stop=True,
                    )
                ext = min(wv["wn"] * 512, wv["valid"] - wv["wb"] * 512)
                nc.scalar.activation(
                    wv["pt"][:, wv["wb"] * 512: wv["wb"] * 512 + ext],
                    ps[:, 0:ext],
                    exp_fn, scale=SCALE,
                )
                for (j, qs, w, col) in wv["pieces"]:
                    if qs == j * 128:  # piece starts on the causal diagonal
                        nc.vector.tensor_mul(
                            wv["pt"][:, col:col + 128],
                            wv["pt"][:, col:col + 128],
                            tri_sb,
                        )

            def emit_pv(wv):
                st = wv["st"]
                blk = wv["blk"]
                if wv["first"]:
                    # PV accumulator for this q-block. The first matmul into
                    # each PSUM bank carries start=True: the bank-wide
                    # has_written clear makes every other chunk's first write
                    # overwrite-where-clear and later ones accumulate.
                    # Explicit deps pin each bank's clear before its sibling
                    # chunks' first writes so Tile cannot reorder them.
                    blk["po"] = popool.tile([128, 1024], F32, tag="po",
                                            name=f"po{wv['h']}_{wv['q0']}")
                po = blk["po"]
                q0 = wv["q0"]
                for (j, qs, w, col) in wv["pieces"]:
                    for cc in range(w // 128):
                        c = (qs + cc * 128 - q0) // 128
                        off = CHUNK_OFF[c]
                        bank = 0 if off < 512 else 1
                        lhsT = wv["pt"][:, col + cc * 128: col + cc * 128 + 128]
                        is_clear = bank not in blk["clr"]
                        mm = nc.tensor.matmul(
                            po[:, off:off + VW],
                            lhsT,
                            st["vA"][:, j * VW:(j + 1) * VW],
                            start=is_clear,
                            stop=False, skip_group_check=True,
                        )
                        if is_clear:
                            blk["clr"][bank] = mm.ins
                        elif c not in blk["cwr"]:
                            add_dep_helper(mm.ins, blk["clr"][bank], sync=False,
                                           reason="bank clear before sibling writes")
                        blk["cwr"].add(c)
                if wv["last"]:
                    # Drain po with two copies (frees both banks), then
                    # normalize from SBUF off the critical path.
                    h, nch = wv["h"], wv["qw"] // 128
                    un = unpool.tile([128, 1024], F32, tag="un",
                                     name=f"un{h}_{q0}")
                    nc.vector.tensor_copy(un[:, 0:CHUNK_OFF[2] + VW],
                                          po[:, 0:CHUNK_OFF[2] + VW])
                    wB = (nch - 3) * 129
                    nc.vector.tensor_copy(un[:, 512:512 + wB],
                                          po[:, 512:512 + wB])
                    for c in range(nch):
                        qi = q0 // 128 + c
                        rc = npool.tile([128, 1], F32, tag="rc",
                                        name=f"rc{h}_{qi}")
                        nc.vector.reciprocal(
                            rc, un[:, CHUNK_OFF[c] + D: CHUNK_OFF[c] + D + 1]
                        )
                        nc.vector.tensor_scalar_mul(
                            st["out"][:, qi * D:(qi + 1) * D],
                            un[:, CHUNK_OFF[c]:CHUNK_OFF[c] + D],
                            rc,
                        )
                    if h == hpc - 1 and q0 == 0:
                        # Last head runs blocks {768, 0, 1536}: after the
                        # second block's normalize, q columns [0, 1536) are
                        # ready. Ship them early so the final DMA is small.
                        nc.sync.dma_start(out=out[h][:, 0:12 * D],
                                          in_=st["out"][:, 0:12 * D])
                    if wv["head_last"]:
                        if h == hpc - 1:
                            nc.sync.dma_start(out=out[h][:, 12 * D:],
                                              in_=st["out"][:, 12 * D:])
                        else:
                            nc.sync.dma_start(out=out[h], in_=st["out"])

            for h in range(hpc):
                # Tiny first-wave slices first (kT[:, :128] covers head 0's
                # single-bank first wave, then the rest of the first wave
                # group and the first block's qT columns), then the bulk.
                w0k = wave_banks * 128
                q0c = BLOCKS[0][0]
                kT_sb = iopool.tile([128, s], BF16, tag="kT", name=f"kT{h}")
                if h == 0:
                    nc.sync.dma_start(out=kT_sb[:, 0:128], in_=kT[h][:, 0:128])
                else:
                    nc.sync.dma_start(out=kT_sb[:, 0:w0k], in_=kT[h][:, 0:w0k])
                qT_sb = iopool.tile([128, s], BF16, tag="qT", name=f"qT{h}")
                nc.sync.dma_start(out=qT_sb[:, q0c:s], in_=qT[h][:, q0c:s])
                vA_sb = iopool.tile([128, nt * VW], BF16, tag="vA", name=f"vA{h}")
                if h == 0:
                    # Stage head 0's transfers in the order the ramp needs
                    # them: wave w covers kv-tiles 3w-2..3w, the first tri
                    # follows exp(wave 0), and the first PV wave follows
                    # ~4 waves later.
                    nc.sync.dma_start(out=kT_sb[:, 128:512],
                                      in_=kT[h][:, 128:512])
                    nc.sync.dma_start(out=kT_sb[:, 512:1024],
                                      in_=kT[h][:, 512:1024])
                    nc.sync.dma_start(out=tri_sb, in_=tri[:, :])
                    nc.sync.dma_start(out=vA_sb[:, 0:8 * VW],
                                      in_=vA[h][:, 0:8 * VW])
                    nc.sync.dma_start(out=kT_sb[:, 1024:s],
                                      in_=kT[h][:, 1024:s])
                    nc.sync.dma_start(out=vA_sb[:, 8 * VW:],
                                      in_=vA[h][:, 8 * VW:])
                else:
                    nc.sync.dma_start(out=kT_sb[:, w0k:s], in_=kT[h][:, w0k:s])
                    nc.sync.dma_start(out=vA_sb, in_=vA[h])
                nc.sync.dma_start(out=qT_sb[:, 0:q0c], in_=qT[h][:, 0:q0c])
                out_sb = iopool.tile([128, nt * D], F32, tag="osb", name=f"osb{h}")
                st = {"kT": kT_sb, "qT": qT_sb, "vA": vA_sb, "out": out_sb}

                # Last head ends on the 4-chunk 512-wide block so the
                # end-of-kernel normalize + store chain is minimal, but
                # still opens with a deep 5-wave block.
                if h == hpc - 1:
                    blocks_h = [BLOCKS[1], BLOCKS[2], BLOCKS[0]]
                else:
                    blocks_h = BLOCKS
                for bi, (q0, qw) in enumerate(blocks_h):
                    pieces, nbanks, valid = _block_layout(q0, qw)
                    pt_sb = ptpool.tile([128, ptw], BF16, tag="pt",
                                        name=f"pt{h}_{q0}")
                    blk = {"po": None, "clr": {}, "cwr": set()}
                    wb = 0
                    qwaves = []
                    first_wave = True
                    while wb < nbanks:
                        wn = min(wave_banks, nbanks - wb)
                        if h == 0 and bi == 0 and first_wave:
                            # Single-bank first wave: the very first exp only
                            # needs one bank of scores, so ACT starts ~2us
                            # sooner after the initial DMA.
                            wn = 1
                        qwaves.append({
                            "h": h, "q0": q0, "qw": qw, "wb": wb, "wn": wn,
                            "pieces": [p for p in pieces
                                       if wb * 512 <= p[3] < (wb + wn) * 512],
                            "valid": valid, "pt": pt_sb, "st": st, "blk": blk,
                            "first": wb == 0, "last": False,
                            "head_last": False,
                        })
                        wb += wn
                        first_wave = False
                    qwaves[-1]["last"] = True
                    qwaves[-1]["head_last"] = bi == len(BLOCKS) - 1
                    for wv in qwaves:
                        emit_scores(wv)
                        pending.append(wv)
                        while len(pending) > pv_lag_target():
                            emit_pv(pending.pop(0))
            for wv in pending:
                emit_pv(wv)
    nc.compile()
    return nc


def _pack_inputs(xq, xk, xv, s=S, b=B, h=H):
    """Full [B,S,H,D] fp32 inputs -> per-pair device layouts (bf16)."""
    bf16 = ml_dtypes.bfloat16
    nt = s // 128
    nh = b * h
    # [B,S,H,D] -> [B,H,S,D] -> [nh, S, D]
    q = np.transpose(np.asarray(xq), (0, 2, 1, 3)).reshape(nh, s, D)
    k = np.transpose(np.asarray(xk), (0, 2, 1, 3)).reshape(nh, s, D)
    v = np.transpose(np.asarray(xv), (0, 2, 1, 3)).reshape(nh, s, D)
    qT = np.ascontiguousarray(q.transpose(0, 2, 1)).astype(bf16)  # [nh, D, S]
    kT = np.ascontiguousarray(k.transpose(0, 2, 1)).astype(bf16)
    v4 = v.reshape(nh, nt, 128, D)
    ones = np.ones((nh, nt, 128, 1), np.float32)
    vA = np.concatenate([v4, ones], axis=3)          # [nh, nt, 128, VW]
    vA = np.ascontiguousarray(vA.transpose(0, 2, 1, 3)).reshape(nh, 128, nt * VW)
    vA = vA.astype(bf16)
    tri = np.triu(np.ones((128, 128), np.float32)).astype(bf16)
    return qT, kT, vA, tri


def _unpack_output(outs, s=S, b=B, h=H):
    """Per-core [hpc, 128, NT*D] fp32 -> [B, S, H*D]."""
    nt = s // 128
    o = np.concatenate([np.asarray(x) for x in outs], axis=0)  # [nh, 128, nt*D]
    o = o.reshape(b * h, 128, nt, D).transpose(0, 2, 1, 3)     # [nh, nt, 128, D]
    o = o.reshape(b, h, s, D).transpose(0, 2, 1, 3)            # [B, S, H, D]
    return np.ascontiguousarray(o.reshape(b, s, h * D)).astype(np.float32)


_CACHE = {}


def _get_module():
    if "nc" not in _CACHE:
        _CACHE["nc"] = build_module()
    return _CACHE["nc"]


def make_in_maps(xq, xk, xv):
    qT, kT, vA, tri = _pack_inputs(xq, xk, xv)
    in_maps = []
    for core in range(N_CORES):
        sl = slice(core * HPC, (core + 1) * HPC)
        in_maps.append({
            "qT": np.ascontiguousarray(qT[sl]),
            "kT": np.ascontiguousarray(kT[sl]),
            "vA": np.ascontiguousarray(vA[sl]),
            "tri": tri,
        })
    return in_maps


def kernel(xq, xk, xv, cache_k, cache_v, mask, start_pos):
    assert int(start_pos) == 0, "kernel specialized for start_pos == 0"
    from concourse.bass_utils import run_bass_kernel_spmd

    nc = _get_module()
    in_maps = make_in_maps(xq, xk, xv)
    res = None
    for attempt in range(3):
        try:
            res = run_bass_kernel_spmd(nc, in_maps, core_ids=list(range(N_CORES)))
            break
        except Exception:
            if attempt == 2:
                raise
    outs = [res.results[i]["out"] for i in range(N_CORES)]
    return _unpack_output(outs)

